# revision 8
# baseline (speedup 1.0000x reference)
"""Trainium2 Bass kernel for nn_DiagLrMGreen (diagonal-in-k low-rank mixer).

Math: out[b,o,k] = sum_i x[b,i,k] * W[i,o,k] with
      W[i,o,k] = sum_h (U_in[:,:,k,h] @ M[:,:,k,h] @ U_out[:,:,k,h].T)[i,o]

W is precombined on the host (cheap, ~2 GFLOP) — this leaves the device
kernel as a pure batched-small-matmul stream with the same total HBM
traffic as streaming the raw factors (32 MB/core vs 33 MB/core), i.e. the
memory roofline is unchanged while the device work becomes regular.

Sharding: modes axis k split across 8 cores (1024 modes each), zero
communication. Per core, modes are processed in pairs via one K=128
matmul per pair: lhsT is a 128x64 block-diagonal [x_k1 | x_k2] tile
(zeros memset once, DMAs refill only the diagonal blocks), rhs is the
stacked [W_k1 ; W_k2]. Output lands in PSUM [64, 64] per pair
(partitions = (half, b)), is copied to SBUF by the vector engine and
DMA'd out on the gpsimd SWDGE ring while the sync- and scalar-engine
HWDGE rings alternate streaming input chunks.
"""

from contextlib import ExitStack

import numpy as np

import concourse.bass as bass
import concourse.mybir as mybir
from concourse.bass_utils import run_bass_kernel_spmd

NCORES = 8
KTOT = 8192
KLOC = KTOT // NCORES  # 1024 modes per core
NCH = 4                # chunks per core
CH = KLOC // NCH       # 256 modes per chunk
NPAIR = CH // 2        # 128 mode-pairs per chunk
NBANK = 8              # psum banks
NF = NPAIR // 16       # psum bank fills per chunk (8)
B, I, O = 32, 64, 64

F32 = mybir.dt.float32
F16 = mybir.dt.float16

_cache = {}

# fp16 weights are pre-scaled by WSCALE on the host (keeps the tiny W
# entries well inside fp16 normal range); the output is divided back in
# _unpack_out.
WSCALE = 64.0
IN_DT = F16
IN_NP = np.float16


def _build_bass(niter=1):
    nc = bass.Bass("TRN2", target_bir_lowering=False, debug=False,
                   num_devices=NCORES)

    xin = nc.dram_tensor("xin", [NCH, 128, NPAIR, B], IN_DT, kind="ExternalInput")
    win = nc.dram_tensor("win", [NCH, 128, NPAIR, O], IN_DT, kind="ExternalInput")
    odram = nc.dram_tensor("out", [NCH, 128, NF, 512], F32, kind="ExternalOutput")

    with ExitStack() as ctx:
        xt = [ctx.enter_context(nc.sbuf_tensor(f"xt{j}", [128, NPAIR, B], IN_DT))
              for j in range(2)]
        wt = [ctx.enter_context(nc.sbuf_tensor(f"wt{j}", [128, NPAIR, O], IN_DT))
              for j in range(2)]
        ob = [ctx.enter_context(nc.sbuf_tensor(f"ob{j}", [128, NF, 512], F32))
              for j in range(2)]
        pt = [ctx.enter_context(nc.psum_tensor(f"pt{j}", [128, 512], F32))
              for j in range(NBANK)]

        sem_in_sp = ctx.enter_context(nc.semaphore("sem_in_sp"))
        sem_in_act = ctx.enter_context(nc.semaphore("sem_in_act"))
        sem_mm = ctx.enter_context(nc.semaphore("sem_mm"))
        sem_cp = ctx.enter_context(nc.semaphore("sem_cp"))
        sem_out = ctx.enter_context(nc.semaphore("sem_out"))

        def in_chunks(eng, sem, parity):
            ci = 0
            for gc in range(parity, NCH * niter, 2):
                c = gc % NCH
                j = gc % 2
                if gc >= 2:
                    # PE must be done reading slot j (chunk gc-2)
                    eng.wait_ge(sem_mm, NF * (gc - 1))
                eng.dma_start(xt[j][:, :, :], xin[c]).then_inc(sem, 16)
                eng.dma_start(wt[j][:, :, :], win[c]).then_inc(sem, 16)
                ci += 1
                # post-chunk self-wait: race-detector ordering + issue throttle
                eng.wait_ge(sem, 32 * ci)

        with nc.Block() as block:

            @block.sync
            def _(sync):
                in_chunks(sync, sem_in_sp, 0)

            @block.scalar
            def _(scalar):
                in_chunks(scalar, sem_in_act, 1)

            @block.tensor
            def _(tensor):
                for gc in range(NCH * niter):
                    j = gc % 2
                    insem = sem_in_sp if gc % 2 == 0 else sem_in_act
                    tensor.wait_ge(insem, 32 * (gc // 2 + 1))
                    if gc >= 1:
                        tensor.wait_ge(sem_mm, NF * gc)  # self-ordering
                    for u in range(NPAIR // 2):
                        bic, s = u // 8, u % 8
                        fill = NF * gc + bic
                        T = pt[fill % NBANK]
                        if s == 0 and fill >= NBANK:
                            # DVE must have drained this psum tile
                            tensor.wait_ge(sem_cp, fill - NBANK + 1)
                        cs = slice(s * 64, (s + 1) * 64)
                        mm = None
                        for p2 in range(2):
                            g = 2 * u + p2
                            tensor.matmul(
                                T[64 * p2:64 * p2 + 32, cs],
                                xt[j][0:64, g, :],
                                wt[j][0:64, g, :],
                                start=True, stop=True,
                                tile_position=(0, 64 * p2),
                            )
                            mm = tensor.matmul(
                                T[64 * p2 + 32:64 * p2 + 64, cs],
                                xt[j][64:128, g, :],
                                wt[j][64:128, g, :],
                                start=True, stop=True,
                                tile_position=(64, 64 * p2 + 32),
                            )
                        if s == 7:
                            mm.then_inc(sem_mm, 1)

            @block.vector
            def _(vector):
                for gc in range(NCH * niter):
                    j = gc % 2
                    if gc >= 1:
                        vector.wait_ge(sem_cp, NF * gc)  # self-ordering
                    if gc >= 2:
                        # out-DMA must be done with ob slot j (chunk gc-2)
                        vector.wait_ge(sem_out, 16 * (gc - 1))
                    for bic in range(NF):
                        fill = NF * gc + bic
                        vector.wait_ge(sem_mm, fill + 1)
                        vector.tensor_copy(ob[j][:, bic, :], pt[fill % NBANK][:, :]).then_inc(sem_cp, 1)

            @block.gpsimd
            def _(gpsimd):
                for gc in range(NCH * niter):
                    c = gc % NCH
                    if gc >= 1:
                        gpsimd.wait_ge(sem_out, 16 * gc)  # self-ordering
                    gpsimd.wait_ge(sem_cp, NF * (gc + 1))
                    gpsimd.dma_start(odram[c], ob[gc % 2][:]).then_inc(sem_out, 16)

    return nc


def _combine_w(U_in, M, U_out):
    # W[k,i,o] = sum_h U_in[:,:,k,h] @ M[:,:,k,h] @ U_out[:,:,k,h].T
    Ui = np.ascontiguousarray(U_in.transpose(2, 3, 0, 1))  # [k,h,i,r]
    Mm = np.ascontiguousarray(M.transpose(2, 3, 0, 1))     # [k,h,r,s]
    Uo = np.ascontiguousarray(U_out.transpose(2, 3, 1, 0)) # [k,h,s,o]
    T = np.matmul(Ui, Mm)                                  # [k,h,i,s]
    W = np.matmul(T, Uo).sum(axis=1)                       # [k,i,o]
    return np.ascontiguousarray(W, dtype=np.float32)


def _pack_core(xs, Ws):
    """xs: [B, I, KLOC] fp32, Ws: [KLOC, I, O] fp32 -> (xin, win) arrays."""
    # k_local = c*CH + 2*g + half
    x5 = xs.reshape(B, I, NCH, NPAIR, 2)          # [b,i,c,g,half]
    xin = np.ascontiguousarray(x5.transpose(2, 4, 1, 3, 0), dtype=IN_NP)
    xin = xin.reshape(NCH, 128, NPAIR, B)
    # win[c, half*64+i, g, o]
    w5 = (Ws * WSCALE).reshape(NCH, NPAIR, 2, I, O)  # [c,g,half,i,o]
    win = np.ascontiguousarray(w5.transpose(0, 2, 3, 1, 4), dtype=IN_NP)
    return xin, win.reshape(NCH, 128, NPAIR, O)


def _unpack_out(od):
    """od: [NCH, 128, 4, 512] -> [B, O, KLOC]"""
    # partitions = p2*64 + half*32 + b; free = bic*512 + s*64 + o
    o7 = od.reshape(NCH, 2, 2, B, NF, 8, O)       # [c,p2,half,b,bic,s,o]
    # k_local = c*CH + bic*32 + s*4 + p2*2 + half
    out = o7.transpose(3, 6, 0, 4, 5, 1, 2).reshape(B, O, KLOC)
    return out * np.float32(1.0 / WSCALE) if WSCALE != 1.0 else out


def kernel(x, U_in, M, U_out):
    x = np.asarray(x, dtype=np.float32)
    W = _combine_w(np.asarray(U_in, dtype=np.float32),
                   np.asarray(M, dtype=np.float32),
                   np.asarray(U_out, dtype=np.float32))

    if "nc" not in _cache:
        _cache["nc"] = _build_bass()
    nc = _cache["nc"]

    in_maps = []
    for cid in range(NCORES):
        k0 = cid * KLOC
        xin, win = _pack_core(x[:, :, k0:k0 + KLOC], W[k0:k0 + KLOC])
        in_maps.append({"xin": xin, "win": win})

    res = run_bass_kernel_spmd(nc, in_maps, list(range(NCORES)))

    out = np.empty((B, O, KTOT), dtype=np.float32)
    for cid in range(NCORES):
        k0 = cid * KLOC
        out[:, :, k0:k0 + KLOC] = _unpack_out(res.results[cid]["out"])
    return out


# revision 11
# speedup vs baseline: 1.4385x; 1.4385x over previous
"""Trainium2 Bass kernel for nn_DiagLrMGreen (diagonal-in-k low-rank mixer).

Math: out[b,o,k] = sum_i x[b,i,k] * W[i,o,k] with
      W[i,o,k] = sum_h (U_in[:,:,k,h] @ M[:,:,k,h] @ U_out[:,:,k,h].T)[i,o]

W is precombined on the host (cheap, ~2 GFLOP) — this leaves the device
kernel as a pure batched-small-matmul stream with the same total HBM
traffic as streaming the raw factors (32 MB/core vs 33 MB/core), i.e. the
memory roofline is unchanged while the device work becomes regular.

Sharding: modes axis k split across 8 cores (1024 modes each), zero
communication. Per core, modes are processed in pairs (two modes share
the 128 SBUF partitions: mode A on partitions 0:63, mode B on 64:127).
Each mode is one small matmul (K=64 contraction over i, M=32 batch
columns, N=64 out channels); four modes run CONCURRENTLY on the PE via
tile_position row/col packing (rows {0,64} x cols {0,32,64,96}), each
writing its own 32-partition slice of a [128, 512] PSUM bank. Inputs are
fp16 (x as-is; W pre-scaled by WSCALE to sit in fp16 normal range),
PSUM accumulates fp32, output is returned fp32 — this cuts HBM traffic
from 32 MB/core (fp32) to 20 MB/core at ~1.7e-4 relative error. Banks
are copied PSUM->SBUF by the vector engine and DMA'd out on the gpsimd
SWDGE ring while the sync- and scalar-engine HWDGE rings alternate
streaming input chunks; every DMA is fully contiguous on both sides.
All semaphore waits are emitted one-per-instruction (this walrus build
rejects multi-wait sync_info), and every producer self-waits its own
semaphore at chunk boundaries to satisfy the CoreSim race detector's
update-crossing-an-armed-wait rule.
"""

from contextlib import ExitStack

import numpy as np

import concourse.bass as bass
import concourse.mybir as mybir
from concourse.bass_utils import run_bass_kernel_spmd

NCORES = 8
KTOT = 8192
KLOC = KTOT // NCORES  # 1024 modes per core
NCH = 4                # chunks per core
CH = KLOC // NCH       # 256 modes per chunk
NPAIR = CH // 2        # 128 mode-pairs per chunk
NBANK = 8              # psum banks
NF = NPAIR // 16       # psum bank fills per chunk (8)
B, I, O = 32, 64, 64

F32 = mybir.dt.float32
F16 = mybir.dt.float16

_cache = {}

# fp16 weights are pre-scaled by WSCALE on the host (keeps the tiny W
# entries well inside fp16 normal range); the output is divided back in
# _unpack_out.
WSCALE = 64.0
IN_DT = F16
IN_NP = np.float16


def _build_bass(niter=1):
    nc = bass.Bass("TRN2", target_bir_lowering=False, debug=False,
                   num_devices=NCORES)

    xin = nc.dram_tensor("xin", [NCH, 128, NPAIR, B], IN_DT, kind="ExternalInput")
    win = nc.dram_tensor("win", [NCH, 128, NPAIR, O], IN_DT, kind="ExternalInput")
    odram = nc.dram_tensor("out", [NCH, 128, NF, 512], F32, kind="ExternalOutput")

    with ExitStack() as ctx:
        xt = [ctx.enter_context(nc.sbuf_tensor(f"xt{j}", [128, NPAIR, B], IN_DT))
              for j in range(2)]
        wt = [ctx.enter_context(nc.sbuf_tensor(f"wt{j}", [128, NPAIR, O], IN_DT))
              for j in range(2)]
        ob = [ctx.enter_context(nc.sbuf_tensor(f"ob{j}", [128, NF, 512], F32))
              for j in range(2)]
        pt = [ctx.enter_context(nc.psum_tensor(f"pt{j}", [128, 512], F32))
              for j in range(NBANK)]

        sem_in_sp = ctx.enter_context(nc.semaphore("sem_in_sp"))
        sem_in_act = ctx.enter_context(nc.semaphore("sem_in_act"))
        sem_mm = ctx.enter_context(nc.semaphore("sem_mm"))
        sem_cp = ctx.enter_context(nc.semaphore("sem_cp"))
        sem_out = ctx.enter_context(nc.semaphore("sem_out"))

        def in_chunks(eng, sem, parity):
            ci = 0
            for gc in range(parity, NCH * niter, 2):
                c = gc % NCH
                j = gc % 2
                if gc >= 2:
                    # PE must be done reading slot j (chunk gc-2)
                    eng.wait_ge(sem_mm, NF * (gc - 1))
                eng.dma_start(xt[j][:, :, :], xin[c]).then_inc(sem, 16)
                eng.dma_start(wt[j][:, :, :], win[c]).then_inc(sem, 16)
                ci += 1
                # post-chunk self-wait: race-detector ordering + issue throttle
                eng.wait_ge(sem, 32 * ci)

        with nc.Block() as block:

            @block.sync
            def _(sync):
                in_chunks(sync, sem_in_sp, 0)

            @block.scalar
            def _(scalar):
                in_chunks(scalar, sem_in_act, 1)

            @block.tensor
            def _(tensor):
                for gc in range(NCH * niter):
                    j = gc % 2
                    insem = sem_in_sp if gc % 2 == 0 else sem_in_act
                    tensor.wait_ge(insem, 32 * (gc // 2 + 1))
                    if gc >= 1:
                        tensor.wait_ge(sem_mm, NF * gc)  # self-ordering
                    for u in range(NPAIR // 2):
                        bic, s = u // 8, u % 8
                        fill = NF * gc + bic
                        T = pt[fill % NBANK]
                        if s == 0 and fill >= NBANK:
                            # DVE must have drained this psum tile
                            tensor.wait_ge(sem_cp, fill - NBANK + 1)
                        cs = slice(s * 64, (s + 1) * 64)
                        mm = None
                        for p2 in range(2):
                            g = 2 * u + p2
                            tensor.matmul(
                                T[64 * p2:64 * p2 + 32, cs],
                                xt[j][0:64, g, :],
                                wt[j][0:64, g, :],
                                start=True, stop=True,
                                tile_position=(0, 64 * p2),
                            )
                            mm = tensor.matmul(
                                T[64 * p2 + 32:64 * p2 + 64, cs],
                                xt[j][64:128, g, :],
                                wt[j][64:128, g, :],
                                start=True, stop=True,
                                tile_position=(64, 64 * p2 + 32),
                            )
                        if s == 7:
                            mm.then_inc(sem_mm, 1)

            @block.vector
            def _(vector):
                for gc in range(NCH * niter):
                    j = gc % 2
                    if gc >= 1:
                        vector.wait_ge(sem_cp, NF * gc)  # self-ordering
                    if gc >= 2:
                        # out-DMA must be done with ob slot j (chunk gc-2)
                        vector.wait_ge(sem_out, 16 * (gc - 1))
                    for bic in range(NF):
                        fill = NF * gc + bic
                        vector.wait_ge(sem_mm, fill + 1)
                        vector.tensor_copy(ob[j][:, bic, :], pt[fill % NBANK][:, :]).then_inc(sem_cp, 1)

            @block.gpsimd
            def _(gpsimd):
                for gc in range(NCH * niter):
                    c = gc % NCH
                    if gc >= 1:
                        gpsimd.wait_ge(sem_out, 16 * gc)  # self-ordering
                    gpsimd.wait_ge(sem_cp, NF * (gc + 1))
                    gpsimd.dma_start(odram[c], ob[gc % 2][:]).then_inc(sem_out, 16)

    return nc


def _combine_w(U_in, M, U_out):
    # W[k,i,o] = sum_h U_in[:,:,k,h] @ M[:,:,k,h] @ U_out[:,:,k,h].T
    Ui = np.ascontiguousarray(U_in.transpose(2, 3, 0, 1))  # [k,h,i,r]
    Mm = np.ascontiguousarray(M.transpose(2, 3, 0, 1))     # [k,h,r,s]
    Uo = np.ascontiguousarray(U_out.transpose(2, 3, 1, 0)) # [k,h,s,o]
    T = np.matmul(Ui, Mm)                                  # [k,h,i,s]
    W = np.matmul(T, Uo).sum(axis=1)                       # [k,i,o]
    return np.ascontiguousarray(W, dtype=np.float32)


def _pack_core(xs, Ws):
    """xs: [B, I, KLOC] fp32, Ws: [KLOC, I, O] fp32 -> (xin, win) arrays."""
    # k_local = c*CH + 2*g + half
    x5 = xs.reshape(B, I, NCH, NPAIR, 2)          # [b,i,c,g,half]
    xin = np.ascontiguousarray(x5.transpose(2, 4, 1, 3, 0), dtype=IN_NP)
    xin = xin.reshape(NCH, 128, NPAIR, B)
    # win[c, half*64+i, g, o]
    w5 = (Ws * WSCALE).reshape(NCH, NPAIR, 2, I, O)  # [c,g,half,i,o]
    win = np.ascontiguousarray(w5.transpose(0, 2, 3, 1, 4), dtype=IN_NP)
    return xin, win.reshape(NCH, 128, NPAIR, O)


def _unpack_out(od):
    """od: [NCH, 128, 4, 512] -> [B, O, KLOC]"""
    # partitions = p2*64 + half*32 + b; free = bic*512 + s*64 + o
    o7 = od.reshape(NCH, 2, 2, B, NF, 8, O)       # [c,p2,half,b,bic,s,o]
    # k_local = c*CH + bic*32 + s*4 + p2*2 + half
    out = o7.transpose(3, 6, 0, 4, 5, 1, 2).reshape(B, O, KLOC)
    return out * np.float32(1.0 / WSCALE) if WSCALE != 1.0 else out


def kernel(x, U_in, M, U_out):
    x = np.asarray(x, dtype=np.float32)
    W = _combine_w(np.asarray(U_in, dtype=np.float32),
                   np.asarray(M, dtype=np.float32),
                   np.asarray(U_out, dtype=np.float32))

    if "nc" not in _cache:
        _cache["nc"] = _build_bass()
    nc = _cache["nc"]

    in_maps = []
    for cid in range(NCORES):
        k0 = cid * KLOC
        xin, win = _pack_core(x[:, :, k0:k0 + KLOC], W[k0:k0 + KLOC])
        in_maps.append({"xin": xin, "win": win})

    res = run_bass_kernel_spmd(nc, in_maps, list(range(NCORES)))

    out = np.empty((B, O, KTOT), dtype=np.float32)
    for cid in range(NCORES):
        k0 = cid * KLOC
        out[:, :, k0:k0 + KLOC] = _unpack_out(res.results[cid]["out"])
    return out
